# revision 21
# baseline (speedup 1.0000x reference)
"""Trainium2 Bass kernel for nn_AttentionTE_15221364097676.

Reference computation (fp32):
    xn  = LayerNorm(x) * ln_w + ln_b
    qkv = xn @ w_qkv.T -> per-head q,k,v (H=16 heads, C=64), q *= C**-0.5
    a   = softmax(q k^T + bias, masked over keys)
    y   = (a @ v).reshape(B,N,D)
    out = (sigmoid(xn @ w_g.T + b_g) * y) @ w_o.T + b_o

Sharding (8 cores): data-parallel over B (cores 0-3 -> b=0, 4-7 -> b=1),
tensor-parallel over heads (4 heads/core).  o_proj is row-parallel; the
4 partial outputs per batch are summed on the host during unsharding
(+ b_o, also host-applied).

Kernel structure (projections fp16, attention fp16/bf16, PSUM f32):
  - token-chunk-pipelined LayerNorm + projections: per 512-token chunk,
    PE ones-matmuls produce sum(x)/sum(x^2) rows (squares on ACT), row
    math on DVE at partition 0, rstd row on ACT (Ln/Exp), PE broadcast,
    DVE normalize, then the chunk's q/k/v projections.  Stats run two
    chunks ahead of projections so the PE never waits on the DVE/ACT
    row pipeline and the HAM clock gate stays warm.
  - the mean term enters the projections as rank-1 matmuls
    (w @ xn = w @ xs - colsum(w) (x) (mu*rstd)); ln_w folded on host.
  - attention (pair-major, 512-wide q chunks): bias injected into PSUM
    by identity matmuls on the PE for 8/16 key tiles and by DVE adds
    for the other 8 (engine balance), two heads' scores row-group
    packed, ACT exp straight from PSUM with the key mask applied via
    its per-partition bias operand, v2.T @ p accumulates y plus a
    denominator column (constant 1s in v2).
  - epilogue without ACT: the denominator row is copied to f32r, PE
    broadcast to 64 partitions, inverted with one DVE
    reciprocal_approx_fast per head, gate-multiplied on DVE (geff kept
    f32: g/den can sit far below the fp16 subnormal cutoff).
  - o_proj (fp16) interleaved into pair-1 attention chunk-by-chunk;
    f32 partial outputs summed on the host.
  - an activation-table shim makes Ln/Exp/Copy/Square resolve to the
    combined natural_log_exp table so the ACT engine loads ~3 tables
    total instead of thrashing between the exp and ln sets.
"""

import sys

for _p in ("/opt/trn_rl_repo",):
    if _p not in sys.path:
        sys.path.insert(0, _p)

from contextlib import ExitStack

import ml_dtypes
import numpy as np

import concourse.bass as bass
import concourse.tile as tile
from concourse import bacc, mybir
from concourse.bass import ds, ts

F32 = mybir.dt.float32
F32R = mybir.dt.float32r
BF16 = mybir.dt.bfloat16
F16 = mybir.dt.float16
AF = mybir.ActivationFunctionType
OP = mybir.AluOpType

B, N, D, H, C = 2, 2048, 1024, 16, 64
HPC = 4          # heads per core
NCORES = 8
DT = D // 128    # 8 d-tiles
NT = N // 128    # 16 token tiles
KT = N // 128    # 16 key tiles
CC = 4           # 512-token chunks
VW = 65          # v2 slice width per head: 64 v-cols + 1 denominator col
EPS = 1e-5
NEG = -1.0e30    # additive key-mask value


def _emit(tc, ctx, io, aug):
    nc = tc.nc

    # ---- long-lived pools ---------------------------------------------------
    const = ctx.enter_context(tc.tile_pool(name="const", bufs=1))
    wts = ctx.enter_context(tc.tile_pool(name="wts", bufs=1))
    bias_pool = ctx.enter_context(tc.tile_pool(name="bias", bufs=3))
    qk_pool = ctx.enter_context(tc.tile_pool(name="qkT", bufs=1))
    v_pool = ctx.enter_context(tc.tile_pool(name="v2", bufs=1))
    g_pool = ctx.enter_context(tc.tile_pool(name="gate", bufs=1))
    yg_pool = ctx.enter_context(tc.tile_pool(name="yg", bufs=1))

    # ---- early DMA issue: x chunks, bias prefetch, weights ------------------
    xt_stack = ExitStack()
    xpool = xt_stack.enter_context(tc.tile_pool(name="xt", bufs=1))
    xt = xpool.tile([128, DT, N], F16)
    for cc in range(2):
        nc.sync.dma_start(
            xt[:, :, ts(cc, 512)],
            io["xT"][cc].rearrange("p (dt q) -> p dt q", dt=DT))

    bts = {}

    def load_bt(chunk, half):
        pair, c4 = chunk // 4, chunk % 4
        bt = bias_pool.tile([128, 8, 2, 512], F16, tag="bt", name="bt")
        nc.sync.dma_start(
            bt[:], io["biasT"][pair, half, c4].rearrange(
                "p (g h q) -> p g h q", g=8, h=2))
        bts[(chunk, half)] = bt

    wqk_sb = wts.tile([128, DT, 512], F16)
    nc.sync.dma_start(wqk_sb[:], io["wqk"].rearrange("(dt p) m -> p dt m", p=128))
    load_bt(0, 0)
    load_bt(0, 1)
    for cc in range(2, CC):
        nc.sync.dma_start(
            xt[:, :, ts(cc, 512)],
            io["xT"][cc].rearrange("p (dt q) -> p dt q", dt=DT))
    wv_sb = wts.tile([128, DT, 256], F16)
    nc.sync.dma_start(wv_sb[:], io["wv"].rearrange("(dt p) m -> p dt m", p=128))
    load_bt(1, 0)

    # ---- constants / weights ------------------------------------------------
    wg_sb = wts.tile([128, DT, 256], F16)
    nc.sync.dma_start(wg_sb[:], io["wg"].rearrange("(dt p) m -> p dt m", p=128))
    wo_sb = const.tile([128, 2, 1024], F16)
    nc.sync.dma_start(wo_sb[:], io["wo"].rearrange("(t p) e -> p t e", p=128))
    bg_sb = wts.tile([128, 2], F32)
    nc.sync.dma_start(bg_sb[:], io["bg"])
    wsall = wts.tile([1, 1024], F32R)
    nc.sync.dma_start(wsall[:], io["wsall"])
    wsqk_sb, wsv_sb, wsg_sb = (wsall[:, 0:512], wsall[:, 512:768],
                               wsall[:, 768:1024])
    id_sb = const.tile([128, 128], F16)
    nc.sync.dma_start(id_sb[:], io["ident"])
    ml_sb = const.tile([128, KT], F32)
    nc.sync.dma_start(ml_sb[:], io["maskln"])

    ones_f = const.tile([128, 128], F32)
    nc.vector.memset(ones_f[:], 1.0)
    ones_fr = const.tile([128, 128], F32R)
    nc.vector.tensor_copy(ones_fr[:], ones_f[:])
    ones_bf = const.tile([128, 128], BF16)
    nc.vector.tensor_copy(ones_bf[:], ones_f[:])
    ones_hf = const.tile([128, 128], F16)
    nc.vector.tensor_copy(ones_hf[:], ones_f[:])
    eps_sb = const.tile([1, 1], F32)
    nc.vector.memset(eps_sb[:], EPS)
    msr = wts.tile([1, N], F32R)
    if aug:
        qkb_sb = wts.tile([1, 512], F32R)
        nc.sync.dma_start(qkb_sb[:], io["qkb"])
        vb_sb = wts.tile([1, 256], F32R)
        nc.sync.dma_start(vb_sb[:], io["vb"])
        ones_row_f = wts.tile([1, 512], F32)
        nc.vector.memset(ones_row_f[:], 1.0)
        ones_row = wts.tile([1, 512], F32R)
        nc.vector.tensor_copy(ones_row[:], ones_row_f[:])

    qkT = qk_pool.tile([128, 4, N], F16)
    # v2 per (kt, pair): [vA(0:64) | 1 | vB(65:129) | 1] -- the constant
    # columns put the softmax denominator in row 64 of each y accumulator.
    v2 = v_pool.tile([128, KT, 2, 2 * VW], BF16)
    nc.vector.memset(v2[:], 1.0)
    g_sb = g_pool.tile([128, 2, N], F16)
    gB_sb = g_pool.tile([128, 2, N], F16)
    yg = yg_pool.tile([128, 2, N], F16)

    # ---- Phase A: per-chunk LN + q/k/v projections --------------------------
    with tc.tile_pool(name="sq", bufs=4) as sqpool, \
         tc.tile_pool(name="rows", bufs=2) as rowp, \
         tc.tile_pool(name="rbc", bufs=2) as rbcp, \
         tc.tile_pool(name="stps", bufs=2, space="PSUM") as stps, \
         tc.tile_pool(name="bcps", bufs=1, space="PSUM") as bcps, \
         tc.tile_pool(name="qkps", bufs=3, space="PSUM") as qkps:

        st_of, rstd_of = {}, {}

        def emit_stats(cc):
            # sum(x) and sum(x^2) (squares on GPSIMD), both at partition 0,
            # then row math and the rstd row -- no PE dependency on DVE.
            st_mu = stps.tile([128, 512], F32, tag="mu", name="mu")
            st_sq = stps.tile([128, 512], F32, tag="sqs", name="sqs", bufs=2)
            for dt in range(DT):
                nc.tensor.matmul(st_mu[0:1, :], ones_hf[:, 0:1],
                                 xt[:, dt, ts(cc, 512)],
                                 start=(dt == 0), stop=(dt == DT - 1))
            for dt in range(DT):
                sqt = sqpool.tile([128, 512], F16, tag="sq", name="sq")
                nc.scalar.square(sqt[:], xt[:, dt, ts(cc, 512)])
                nc.tensor.matmul(st_sq[0:1, :], ones_hf[:, 0:1], sqt[:],
                                 start=(dt == 0), stop=(dt == DT - 1))
            rows = rowp.tile([1, 2048], F32, tag="rows", name="rows")
            r_mu, r_mu2 = rows[0:1, 0:512], rows[0:1, 512:1024]
            r_var, r_lnv = rows[0:1, 1024:1536], rows[0:1, 1536:2048]
            nc.vector.tensor_scalar(out=r_mu, in0=st_mu[0:1, :],
                                    scalar1=1.0 / D, scalar2=None, op0=OP.mult)
            nc.vector.tensor_mul(r_mu2, r_mu, r_mu)
            nc.vector.scalar_tensor_tensor(out=r_var, in0=st_sq[0:1, :],
                                           scalar=1.0 / D, in1=r_mu2,
                                           op0=OP.mult, op1=OP.subtract)
            nc.scalar.activation(r_lnv, r_var, AF.Ln,
                                 bias=eps_sb[0:1, 0:1], scale=1.0)
            rstd_bf = rowp.tile([1, 512], F16, tag="rstdb", name="rstdb")
            nc.scalar.activation(rstd_bf[:], r_lnv, AF.Exp, scale=-0.5)
            nc.vector.tensor_tensor(out=msr[0:1, ts(cc, 512)],
                                    in0=r_mu, in1=rstd_bf[:], op=OP.mult)
            rstd_of[cc] = rstd_bf

        def emit_norm(cc):
            # broadcast rstd to 128 partitions, normalize xt in place
            rstd_bf = rstd_of.pop(cc)
            bc = bcps.tile([128, 512], F32, tag="bc", name="bc")
            nc.tensor.matmul(bc[:], ones_hf[0:1, :], rstd_bf[:],
                             start=True, stop=True)
            rbc = rbcp.tile([128, 512], F16, tag="rbc", name="rbc")
            nc.vector.tensor_copy(rbc[:], bc[:])
            for dt in range(DT):
                nc.vector.tensor_mul(xt[:, dt, ts(cc, 512)],
                                     xt[:, dt, ts(cc, 512)], rbc[:])

        def emit_proj(cc):
            # v first (its DVE copies clear the queue early), then q/k
            for nt in range(4 * cc, 4 * cc + 4):
                ps = qkps.tile([128, 512], F32, tag="qk", name="qk")
                for dt in range(DT):
                    nc.tensor.matmul(ps[:, 0:256], xt[:, dt, ts(nt, 128)],
                                     wv_sb[:, dt, :],
                                     start=(dt == 0), stop=False)
                nc.tensor.matmul(ps[:, 0:256], msr[:, ts(nt, 128)],
                                 wsv_sb[:, :],
                                 start=False, stop=(not aug))
                if aug:
                    nc.tensor.matmul(ps[:, 0:256], ones_fr[0:1, :], vb_sb[:],
                                     start=False, stop=True)
                nc.vector.tensor_copy(
                    v2[:, nt].rearrange("q p (b c) -> q p b c", b=2)[:, :, :, 0:64],
                    ps[:, 0:256].rearrange("q (p b c) -> q p b c", p=2, b=2))
            for mt in range(4):
                ps = qkps.tile([128, 512], F32, tag="qk", name="qk")
                for dt in range(DT):
                    nc.tensor.matmul(ps[:], wqk_sb[:, dt, ts(mt, 128)],
                                     xt[:, dt, ts(cc, 512)],
                                     start=(dt == 0), stop=False)
                nc.tensor.matmul(ps[:], wsqk_sb[:, ts(mt, 128)],
                                 msr[:, ts(cc, 512)],
                                 start=False, stop=(not aug))
                if aug:
                    nc.tensor.matmul(ps[:], qkb_sb[:, ts(mt, 128)],
                                     ones_row[:], start=False, stop=True)
                nc.scalar.copy(qkT[:, mt, ts(cc, 512)], ps[:])

        # stats run two chunks ahead of projections so the PE never waits
        # on the DVE/ACT row pipeline
        emit_stats(0)
        emit_stats(1)
        emit_norm(0)
        emit_stats(2)
        emit_proj(0)
        emit_norm(1)
        emit_stats(3)
        emit_proj(1)
        emit_norm(2)
        emit_proj(2)
        emit_norm(3)
        emit_proj(3)

        # ---- Phase B: gate = sigmoid(wg @ xn + bg) -------------------------
        for gt in range(2):
            for cc in range(CC):
                ps = qkps.tile([128, 512], F32, tag="qk", name="qk")
                for dt in range(DT):
                    nc.tensor.matmul(ps[:], wg_sb[:, dt, ts(gt, 128)],
                                     xt[:, dt, ts(cc, 512)],
                                     start=(dt == 0), stop=False)
                nc.tensor.matmul(ps[:], wsg_sb[:, ts(gt, 128)],
                                 msr[:, ts(cc, 512)],
                                 start=False, stop=True)
                nc.scalar.activation(g_sb[:, gt, ts(cc, 512)], ps[:],
                                     AF.Sigmoid, bias=bg_sb[:, gt:gt + 1],
                                     scale=1.0)
    xt_stack.close()

    # head-B gate halves moved to partitions 0..63 (for base-0 epilogues)
    nc.sync.dma_start(gB_sb[0:64, :, :], g_sb[64:128, :, :])

    # ---- Phase C: attention -------------------------------------------------
    att = ExitStack()
    den_pool = att.enter_context(tc.tile_pool(name="den", bufs=1))
    ycp_pool = att.enter_context(tc.tile_pool(name="ycp", bufs=4))
    p_pool = att.enter_context(tc.tile_pool(name="pexp", bufs=4))
    row_pool = att.enter_context(tc.tile_pool(name="erow", bufs=2))
    ygt_pool = att.enter_context(tc.tile_pool(name="ygt", bufs=2))
    ot_pool = att.enter_context(tc.tile_pool(name="ot", bufs=1))
    yp_pool = att.enter_context(tc.tile_pool(name="yps", bufs=2, space="PSUM"))

    # per-(chunk, head) denominator row at partition 64, lane-copied from
    # the y accumulator, inverted in place on the DVE (row slice only).
    rd_of = {}
    ycps_of = {}

    def chunk_body(chunk, sps_pool):
        pair, c4 = chunk // 4, chunk % 4
        qmt, kmt = 2 * pair, 2 * pair + 1
        qlo = c4 * 512
        # prefetch next chunks' bias halves
        nxt = chunk + 1
        if nxt < 8 and (nxt, 1) not in bts:
            load_bt(nxt, 1)
        if nxt + 1 < 8 and (nxt + 1, 0) not in bts:
            load_bt(nxt + 1, 0)
        yp = [yp_pool.tile([128, 512], F32, tag="yp", name="yp")
              for _ in range(2)]
        for kt in range(KT):
            half, gi = kt // 8, kt % 8
            bt = bts[(chunk, half)]
            s_ps = sps_pool.tile([128, 1024], F32, tag="sps", name="sps")
            dve_inject = 6 <= kt <= 13
            if not dve_inject:
                for hh in range(2):
                    nc.tensor.matmul(s_ps[:, ts(hh, 512)], id_sb[:],
                                     bt[:, gi, hh, :],
                                     start=True, stop=False,
                                     skip_group_check=True)
            for h, base in ((0, 0), (1, 64)):
                nc.tensor.matmul(
                    s_ps[:, ts(h, 512)],
                    qkT[base:base + 64, kmt, ts(kt, 128)],
                    qkT[base:base + 64, qmt, ds(qlo, 512)],
                    start=dve_inject, stop=True, skip_group_check=True)
                if dve_inject:
                    nc.vector.tensor_tensor(
                        out=s_ps[:, ts(h, 512)], in0=s_ps[:, ts(h, 512)],
                        in1=bt[:, gi, h, :], op=OP.add)
            p_t = p_pool.tile([128, 1024], BF16, tag="pt", name="pt")
            nc.scalar.activation(p_t[:], s_ps[:], AF.Exp,
                                 bias=ml_sb[:, kt:kt + 1])
            for h in range(2):
                nc.tensor.matmul(yp[h][0:VW, :],
                                 v2[:, kt, pair, ds(h * VW, VW)],
                                 p_t[:, ts(h, 512)],
                                 start=(kt == 0), stop=(kt == KT - 1))
        del bts[(chunk, 0)], bts[(chunk, 1)]
        ycps, rds = [], []
        for h in range(2):
            ycp = ycp_pool.tile([128, 512], F32R, tag="ycp", name="ycp")
            nc.vector.tensor_copy(ycp[0:64, :], yp[h][0:64, :])
            # f32 -> f32r copy rounds the denominator row for the PE
            rd = den_pool.tile([128, 512], F32R, tag="rd", name="rd", bufs=6)
            nc.vector.tensor_copy(rd[64:65, :], yp[h][64:65, :])
            ycps.append(ycp)
            rds.append(rd)
        ycps_of[chunk] = ycps
        rd_of[chunk] = rds

    def emit_epilogue(chunk, rb_pool, rb_tag, rb_shape):
        pair, c4 = chunk // 4, chunk % 4
        qlo = c4 * 512
        ycps = ycps_of.pop(chunk)
        rds = rd_of.pop(chunk)
        for h in range(2):
            # broadcast den to 64 partitions, invert on the DVE (free-size
            # bound, so the [64,512] reciprocal costs the same as a row)
            rb = rb_pool.tile(rb_shape, F32, tag=rb_tag, name=rb_tag)
            nc.tensor.matmul(rb[0:64, 0:512],
                             ones_fr[64:65, 0:64],
                             rds[h][64:65, :],
                             start=True, stop=True)
            rden = row_pool.tile([128, 512], F32, tag="rden", name="rden")
            nc.vector.reciprocal_approx_fast(out=rden[0:64, :],
                                             in_=rb[0:64, 0:512])
            gsl = (g_sb if h == 0 else gB_sb)[0:64, pair, ds(qlo, 512)]
            # f32: geff ~ g/den can sit far below the f16 subnormal cutoff
            geff = row_pool.tile([128, 512], F32, tag="geff", name="geff")
            nc.vector.tensor_tensor(out=geff[0:64, :], in0=rden[0:64, :],
                                    in1=gsl, op=OP.mult)
            if h == 0:
                nc.vector.tensor_tensor(out=yg[0:64, pair, ds(qlo, 512)],
                                        in0=ycps[h][0:64, :],
                                        in1=geff[0:64, :], op=OP.mult)
            else:
                ygt = ygt_pool.tile([128, 512], F16, tag="ygt", name="ygt")
                nc.vector.tensor_tensor(out=ygt[0:64, :],
                                        in0=ycps[h][0:64, :],
                                        in1=geff[0:64, :], op=OP.mult)
                nc.sync.dma_start(yg[64:128, pair, ds(qlo, 512)],
                                  ygt[0:64, :])

    def emit_oproj(c4, op_pool, on_act=False):
        ot = ot_pool.tile([128, 4, 1024], F32, tag="ot", name="ot")
        for j, nt in enumerate(range(4 * c4, 4 * c4 + 4)):
            for half in range(2):
                ps = op_pool.tile([128, 512], F32, tag="op", name="op")
                for pt in range(2):
                    nc.tensor.matmul(ps[:], yg[:, pt, ts(nt, 128)],
                                     wo_sb[:, pt, ds(half * 512, 512)],
                                     start=(pt == 0), stop=(pt == 1))
                if on_act:
                    nc.scalar.copy(ot[:, j, ds(half * 512, 512)], ps[:])
                else:
                    nc.vector.tensor_copy(ot[:, j, ds(half * 512, 512)], ps[:])
            nc.sync.dma_start(io["out_p"][c4][:, ds(j * 1024, 1024)],
                              ot[:, j, :])

    # pair 0: deep scores pipeline (3 bufs); epilogues deferred by one
    # chunk, borrowing an sps buffer for the den broadcast.
    with tc.tile_pool(name="sps0", bufs=3, space="PSUM") as sps0:
        for chunk in range(4):
            chunk_body(chunk, sps0)
            if chunk >= 1:
                emit_epilogue(chunk - 1, sps0, "sps", [128, 1024])
    # pair 1: 2-buf scores pipeline + o_proj PSUM; each o_proj group
    # follows the epilogue that completes its yg columns.
    with tc.tile_pool(name="sps1", bufs=2, space="PSUM") as sps1, \
         tc.tile_pool(name="opps", bufs=2, space="PSUM") as opps:
        for chunk in range(4, 8):
            chunk_body(chunk, sps1)
            emit_epilogue(chunk - 1, opps, "op", [128, 512])
            if chunk >= 5:
                emit_oproj(chunk - 5, opps, on_act=(chunk >= 7))
        emit_epilogue(7, opps, "op", [128, 512])
        emit_oproj(3, opps, on_act=True)
    att.close()


def _patched_act_tables():
    """Activation-table selection shim: remove exp/ln/copy/identity/square
    from every set that precedes natural_log_exp_and_others so those
    functions resolve to the combined set (one ACT table load instead of
    thrashing between exp_and_others and natural_log per epilogue).
    Indices/order of the sets are preserved, so the emitted
    act_func_set_id values still point at the canonical act_info.json
    entries and the loaded tables themselves are unchanged."""
    from concourse.hw_specs import get_activation_tables

    strip = {AF.Exp, AF.Ln, AF.Copy, AF.Identity, AF.Square}

    def patched(arch):
        tabs = get_activation_tables(arch)
        if "natural_log_exp_and_others" not in tabs:
            return tabs
        out, seen = {}, False
        for name, fns in tabs.items():
            if name == "natural_log_exp_and_others":
                seen = True
                out[name] = fns
            elif not seen:
                out[name] = fns - strip
            else:
                out[name] = fns
        return out

    return patched


_CACHED = {}


def build_program(aug=False):
    if aug in _CACHED:
        return _CACHED[aug]
    nc = bacc.Bacc("TRN2", target_bir_lowering=False, debug=False,
                   enable_asserts=False, num_devices=NCORES)
    io = {
        "xT": nc.dram_tensor("xT", (CC, 128, DT * 512), F16,
                             kind="ExternalInput").ap(),
        "wqk": nc.dram_tensor("wqk", (D, 512), F16, kind="ExternalInput").ap(),
        "wv": nc.dram_tensor("wv", (D, 256), F16, kind="ExternalInput").ap(),
        "wg": nc.dram_tensor("wg", (D, 256), F16, kind="ExternalInput").ap(),
        "wo": nc.dram_tensor("wo", (256, D), F16, kind="ExternalInput").ap(),
        "bg": nc.dram_tensor("bg", (128, 2), F32, kind="ExternalInput").ap(),
        "maskln": nc.dram_tensor("maskln", (128, KT), F32,
                                 kind="ExternalInput").ap(),
        "wsall": nc.dram_tensor("wsall", (1, 1024), F32R,
                                kind="ExternalInput").ap(),
        "biasT": nc.dram_tensor("biasT", (2, 2, 4, 128, 8 * 2 * 512), F16,
                                kind="ExternalInput").ap(),
        "ident": nc.dram_tensor("ident", (128, 128), F16,
                                kind="ExternalInput").ap(),
        "out_p": nc.dram_tensor("out_p", (CC, 128, 4 * 1024), F32,
                                kind="ExternalOutput").ap(),
    }
    if aug:
        io["qkb"] = nc.dram_tensor("qkb", (1, 512), F32R,
                                   kind="ExternalInput").ap()
        io["vb"] = nc.dram_tensor("vb", (1, 256), F32R,
                                  kind="ExternalInput").ap()

    import concourse.bacc as bacc_mod
    orig = bacc_mod.get_activation_tables
    bacc_mod.get_activation_tables = _patched_act_tables()
    try:
        with tile.TileContext(nc) as tc, ExitStack() as ctx:
            _emit(tc, ctx, io, aug)
        nc.compile()
    finally:
        bacc_mod.get_activation_tables = orig
    _CACHED[aug] = nc
    return nc


def prep_in_maps(x, bias, mask, ln_w, ln_b, w_qkv, w_o, b_o, w_g, b_g):
    """Host-side sharding: slice/transpose/reorder/cast only (plus exact
    folds of ln_w / ln_b / q-scale into weights, which are O(params))."""
    x = np.asarray(x, np.float32)
    bias = np.asarray(bias, np.float32)
    mask = np.asarray(mask)
    ln_w = np.asarray(ln_w, np.float32)
    ln_b = np.asarray(ln_b, np.float32)
    w_qkv = np.asarray(w_qkv, np.float32)
    w_o = np.asarray(w_o, np.float32)
    w_g = np.asarray(w_g, np.float32)
    b_g = np.asarray(b_g, np.float32)

    wql = w_qkv * ln_w[None, :]          # ln_w fold (exact)
    wgl = w_g * ln_w[None, :]
    qkv_lb = w_qkv @ ln_b                # ln_b rank-1 corrections
    g_lb = w_g @ ln_b
    aug = bool(np.any(ln_b != 0))
    qscale = C ** -0.5
    identity = np.eye(128, dtype=np.float16)

    in_maps = []
    for core in range(NCORES):
        b = core // 4
        h0 = HPC * (core % 4)
        # qk weight Mtiles: [qP0, kP0, qP1, kP1], each [A(64)|B(64)] cols
        qk_rows, qk_scale = [], []
        for pair in range(2):
            hA, hB = h0 + 2 * pair, h0 + 2 * pair + 1
            for off, sc in ((0, qscale), (64, 1.0)):
                for h in (hA, hB):
                    qk_rows.extend(range(h * 192 + off, h * 192 + off + 64))
                    qk_scale.extend([sc] * 64)
        qk_rows = np.array(qk_rows)
        qk_scale = np.array(qk_scale, np.float32)
        v_rows = np.concatenate(
            [np.arange(h * 192 + 128, h * 192 + 192) for h in range(h0, h0 + 4)])
        d0 = 64 * h0

        wqk_c = np.ascontiguousarray(
            (wql[qk_rows] * qk_scale[:, None]).T).astype(np.float16)
        wv_c = np.ascontiguousarray(wql[v_rows].T).astype(np.float16)
        wg_c = np.ascontiguousarray(
            wgl[d0:d0 + 256].T).astype(np.float16)
        wo_c = np.ascontiguousarray(w_o[:, d0:d0 + 256].T).astype(np.float16)
        bg_c = np.ascontiguousarray(
            (b_g + g_lb)[d0:d0 + 256].reshape(2, 128).T)
        mf = mask[b].astype(np.float32)
        maskln_c = np.ascontiguousarray(
            np.where(mf == 0, NEG, 0.0).astype(np.float32).reshape(KT, 128).T)
        # biasT host layout [pair, half, c4, p, g, head, q512]
        bb = bias[b, h0:h0 + 4].reshape(2, 2, 4, 512, 2, 8, 128)
        biasT_c = np.ascontiguousarray(
            bb.transpose(0, 4, 2, 6, 5, 1, 3).reshape(
                2, 2, 4, 128, 8 * 2 * 512)).astype(np.float16)
        # xT chunk-blocked: [cc, p, dt, 512]
        xT = x[b].T  # (D, N)
        xT_c = np.ascontiguousarray(
            xT.reshape(DT, 128, CC, 512).transpose(2, 1, 0, 3).reshape(
                CC, 128, DT * 512)).astype(np.float16)
        # colsum corrections of the bf16-rounded weights, so the rank-1
        # mean term matches what the matmuls actually use
        wsall_c = np.ascontiguousarray(np.concatenate(
            [-wqk_c.astype(np.float32).sum(0),
             -wv_c.astype(np.float32).sum(0),
             -wg_c.astype(np.float32).sum(0)]).reshape(1, 1024))

        im = {
            "xT": xT_c, "wqk": wqk_c, "wv": wv_c, "wg": wg_c, "wo": wo_c,
            "bg": bg_c, "maskln": maskln_c,
            "biasT": biasT_c, "ident": identity, "wsall": wsall_c,
        }
        if aug:
            im["qkb"] = np.ascontiguousarray(
                (qkv_lb[qk_rows] * qk_scale).reshape(1, 512).astype(np.float32))
            im["vb"] = np.ascontiguousarray(
                qkv_lb[v_rows].reshape(1, 256).astype(np.float32))
        in_maps.append(im)
    return in_maps


def unshard_out(arr):
    """Device out_p [CC, 128, 4*1024] bf16 -> (N, D) f32 partial."""
    a = np.asarray(arr).astype(np.float32)
    return a.reshape(CC, 128, 4, 1024).transpose(0, 2, 1, 3).reshape(N, D)


def gather(results, b_o):
    b_o = np.asarray(b_o, np.float32)
    out = np.zeros((B, N, D), np.float32)
    for core, res in enumerate(results):
        out[core // 4] += unshard_out(res["out_p"])
    out += b_o[None, None, :]
    return out


def run(inputs, **spmd_kwargs):
    from concourse import bass_utils
    in_maps = prep_in_maps(**inputs)
    nc = build_program(aug="qkb" in in_maps[0])
    res = bass_utils.run_bass_kernel_spmd(
        nc, in_maps, core_ids=list(range(NCORES)), **spmd_kwargs)
    return gather(res.results, inputs["b_o"]), res


def kernel(**inputs) -> np.ndarray:
    out, _ = run(inputs)
    return out


# revision 22
# speedup vs baseline: 1.0244x; 1.0244x over previous
"""Trainium2 Bass kernel for nn_AttentionTE_15221364097676.

Reference computation (fp32):
    xn  = LayerNorm(x) * ln_w + ln_b
    qkv = xn @ w_qkv.T -> per-head q,k,v (H=16 heads, C=64), q *= C**-0.5
    a   = softmax(q k^T + bias, masked over keys)
    y   = (a @ v).reshape(B,N,D)
    out = (sigmoid(xn @ w_g.T + b_g) * y) @ w_o.T + b_o

Sharding (8 cores): data-parallel over B (cores 0-3 -> b=0, 4-7 -> b=1),
tensor-parallel over heads (4 heads/core).  o_proj is row-parallel; the
4 partial outputs per batch are summed on the host during unsharding
(+ b_o, also host-applied).

Kernel structure (projections fp16, attention fp16/bf16, PSUM f32):
  - token-chunk-pipelined LayerNorm + projections: per 512-token chunk,
    PE ones-matmuls produce sum(x)/sum(x^2) rows (squares on ACT), row
    math on DVE at partition 0, rstd row on ACT (Ln/Exp), PE broadcast,
    DVE normalize, then the chunk's q/k/v projections.  Stats run two
    chunks ahead of projections so the PE never waits on the DVE/ACT
    row pipeline and the HAM clock gate stays warm.
  - the mean term enters the projections as rank-1 matmuls
    (w @ xn = w @ xs - colsum(w) (x) (mu*rstd)); ln_w folded on host.
  - attention (pair-major, 512-wide q chunks): bias injected into PSUM
    by identity matmuls on the PE for 8/16 key tiles and by DVE adds
    for the other 8 (engine balance), two heads' scores row-group
    packed, ACT exp straight from PSUM with the key mask applied via
    its per-partition bias operand, v2.T @ p accumulates y plus a
    denominator column (constant 1s in v2).
  - epilogue without ACT: the denominator row is copied to f32r, PE
    broadcast to 64 partitions, inverted with one DVE
    reciprocal_approx_fast per head, gate-multiplied on DVE (geff kept
    f32: g/den can sit far below the fp16 subnormal cutoff).
  - o_proj (fp16) interleaved into pair-1 attention chunk-by-chunk;
    f32 partial outputs summed on the host.
  - an activation-table shim makes Ln/Exp/Copy/Square resolve to the
    combined natural_log_exp table so the ACT engine loads ~3 tables
    total instead of thrashing between the exp and ln sets.
"""

import sys

for _p in ("/opt/trn_rl_repo",):
    if _p not in sys.path:
        sys.path.insert(0, _p)

from contextlib import ExitStack

import ml_dtypes
import numpy as np

import concourse.bass as bass
import concourse.tile as tile
from concourse import bacc, mybir
from concourse.bass import ds, ts

F32 = mybir.dt.float32
F32R = mybir.dt.float32r
BF16 = mybir.dt.bfloat16
F16 = mybir.dt.float16
AF = mybir.ActivationFunctionType
OP = mybir.AluOpType

B, N, D, H, C = 2, 2048, 1024, 16, 64
HPC = 4          # heads per core
NCORES = 8
DT = D // 128    # 8 d-tiles
NT = N // 128    # 16 token tiles
KT = N // 128    # 16 key tiles
CC = 4           # 512-token chunks
VW = 65          # v2 slice width per head: 64 v-cols + 1 denominator col
EPS = 1e-5
NEG = -1.0e30    # additive key-mask value


def _emit(tc, ctx, io, aug):
    nc = tc.nc

    # ---- long-lived pools ---------------------------------------------------
    const = ctx.enter_context(tc.tile_pool(name="const", bufs=1))
    wts = ctx.enter_context(tc.tile_pool(name="wts", bufs=1))
    bias_pool = ctx.enter_context(tc.tile_pool(name="bias", bufs=3))
    qk_pool = ctx.enter_context(tc.tile_pool(name="qkT", bufs=1))
    v_pool = ctx.enter_context(tc.tile_pool(name="v2", bufs=1))
    g_pool = ctx.enter_context(tc.tile_pool(name="gate", bufs=1))
    yg_pool = ctx.enter_context(tc.tile_pool(name="yg", bufs=1))

    # ---- early DMA issue: x chunks, bias prefetch, weights ------------------
    xt_stack = ExitStack()
    xpool = xt_stack.enter_context(tc.tile_pool(name="xt", bufs=1))
    xt = xpool.tile([128, DT, N], F16)
    for cc in range(2):
        nc.sync.dma_start(
            xt[:, :, ts(cc, 512)],
            io["xT"][cc].rearrange("p (dt q) -> p dt q", dt=DT))

    bts = {}

    def load_bt(chunk, half):
        pair, c4 = chunk // 4, chunk % 4
        bt = bias_pool.tile([128, 8, 2, 512], F16, tag="bt", name="bt")
        nc.sync.dma_start(
            bt[:], io["biasT"][pair, half, c4].rearrange(
                "p (g h q) -> p g h q", g=8, h=2))
        bts[(chunk, half)] = bt

    wqk_sb = wts.tile([128, DT, 512], F16)
    nc.sync.dma_start(wqk_sb[:], io["wqk"].rearrange("(dt p) m -> p dt m", p=128))
    load_bt(0, 0)
    load_bt(0, 1)
    for cc in range(2, CC):
        nc.sync.dma_start(
            xt[:, :, ts(cc, 512)],
            io["xT"][cc].rearrange("p (dt q) -> p dt q", dt=DT))
    wv_sb = wts.tile([128, DT, 256], F16)
    nc.sync.dma_start(wv_sb[:], io["wv"].rearrange("(dt p) m -> p dt m", p=128))
    load_bt(1, 0)

    # ---- constants / weights ------------------------------------------------
    wg_sb = wts.tile([128, DT, 256], F16)
    nc.sync.dma_start(wg_sb[:], io["wg"].rearrange("(dt p) m -> p dt m", p=128))
    wo_sb = const.tile([128, 2, 1024], F16)
    nc.sync.dma_start(wo_sb[:], io["wo"].rearrange("(t p) e -> p t e", p=128))
    bg_sb = wts.tile([128, 2], F32)
    nc.sync.dma_start(bg_sb[:], io["bg"])
    wsall = wts.tile([1, 1024], F32R)
    nc.sync.dma_start(wsall[:], io["wsall"])
    wsqk_sb, wsv_sb, wsg_sb = (wsall[:, 0:512], wsall[:, 512:768],
                               wsall[:, 768:1024])
    id_sb = const.tile([128, 128], F16)
    nc.sync.dma_start(id_sb[:], io["ident"])
    ml_sb = const.tile([128, KT], F32)
    nc.sync.dma_start(ml_sb[:], io["maskln"])

    ones_f = const.tile([128, 128], F32)
    nc.vector.memset(ones_f[:], 1.0)
    ones_fr = const.tile([128, 128], F32R)
    nc.vector.tensor_copy(ones_fr[:], ones_f[:])
    ones_bf = const.tile([128, 128], BF16)
    nc.vector.tensor_copy(ones_bf[:], ones_f[:])
    ones_hf = const.tile([128, 128], F16)
    nc.vector.tensor_copy(ones_hf[:], ones_f[:])
    eps_sb = const.tile([1, 1], F32)
    nc.vector.memset(eps_sb[:], EPS)
    msr = wts.tile([1, N], F32R)
    if aug:
        qkb_sb = wts.tile([1, 512], F32R)
        nc.sync.dma_start(qkb_sb[:], io["qkb"])
        vb_sb = wts.tile([1, 256], F32R)
        nc.sync.dma_start(vb_sb[:], io["vb"])
        ones_row_f = wts.tile([1, 512], F32)
        nc.vector.memset(ones_row_f[:], 1.0)
        ones_row = wts.tile([1, 512], F32R)
        nc.vector.tensor_copy(ones_row[:], ones_row_f[:])

    qkT = qk_pool.tile([128, 4, N], F16)
    # v2 per (kt, pair): [vA(0:64) | 1 | vB(65:129) | 1] -- the constant
    # columns put the softmax denominator in row 64 of each y accumulator.
    v2 = v_pool.tile([128, KT, 2, 2 * VW], BF16)
    nc.vector.memset(v2[:], 1.0)
    g_sb = g_pool.tile([128, 2, N], F16)
    gB_sb = g_pool.tile([128, 2, N], F16)
    yg = yg_pool.tile([128, 2, N], F16)

    # ---- Phase A: per-chunk LN + q/k/v projections --------------------------
    with tc.tile_pool(name="sq", bufs=4) as sqpool, \
         tc.tile_pool(name="rows", bufs=2) as rowp, \
         tc.tile_pool(name="rbc", bufs=2) as rbcp, \
         tc.tile_pool(name="stps", bufs=2, space="PSUM") as stps, \
         tc.tile_pool(name="bcps", bufs=1, space="PSUM") as bcps, \
         tc.tile_pool(name="qkps", bufs=3, space="PSUM") as qkps:

        st_of, rstd_of = {}, {}

        def emit_stats(cc):
            # sum(x) and sum(x^2) (squares on GPSIMD), both at partition 0,
            # then row math and the rstd row -- no PE dependency on DVE.
            st_mu = stps.tile([128, 512], F32, tag="mu", name="mu")
            st_sq = stps.tile([128, 512], F32, tag="sqs", name="sqs", bufs=2)
            for dt in range(DT):
                nc.tensor.matmul(st_mu[0:1, :], ones_hf[:, 0:1],
                                 xt[:, dt, ts(cc, 512)],
                                 start=(dt == 0), stop=(dt == DT - 1))
            for dt in range(DT):
                sqt = sqpool.tile([128, 512], F16, tag="sq", name="sq")
                nc.scalar.square(sqt[:], xt[:, dt, ts(cc, 512)])
                nc.tensor.matmul(st_sq[0:1, :], ones_hf[:, 0:1], sqt[:],
                                 start=(dt == 0), stop=(dt == DT - 1))
            rows = rowp.tile([1, 2048], F32, tag="rows", name="rows")
            r_mu, r_mu2 = rows[0:1, 0:512], rows[0:1, 512:1024]
            r_var, r_lnv = rows[0:1, 1024:1536], rows[0:1, 1536:2048]
            nc.vector.tensor_scalar(out=r_mu, in0=st_mu[0:1, :],
                                    scalar1=1.0 / D, scalar2=None, op0=OP.mult)
            nc.vector.tensor_mul(r_mu2, r_mu, r_mu)
            nc.vector.scalar_tensor_tensor(out=r_var, in0=st_sq[0:1, :],
                                           scalar=1.0 / D, in1=r_mu2,
                                           op0=OP.mult, op1=OP.subtract)
            nc.scalar.activation(r_lnv, r_var, AF.Ln,
                                 bias=eps_sb[0:1, 0:1], scale=1.0)
            rstd_bf = rowp.tile([1, 512], F16, tag="rstdb", name="rstdb")
            nc.scalar.activation(rstd_bf[:], r_lnv, AF.Exp, scale=-0.5)
            nc.vector.tensor_tensor(out=msr[0:1, ts(cc, 512)],
                                    in0=r_mu, in1=rstd_bf[:], op=OP.mult)
            rstd_of[cc] = rstd_bf

        def emit_norm(cc):
            # broadcast rstd to 128 partitions, normalize xt in place
            rstd_bf = rstd_of.pop(cc)
            bc = bcps.tile([128, 512], F32, tag="bc", name="bc")
            nc.tensor.matmul(bc[:], ones_hf[0:1, :], rstd_bf[:],
                             start=True, stop=True)
            rbc = rbcp.tile([128, 512], F16, tag="rbc", name="rbc")
            nc.vector.tensor_copy(rbc[:], bc[:])
            for dt in range(DT):
                nc.vector.tensor_mul(xt[:, dt, ts(cc, 512)],
                                     xt[:, dt, ts(cc, 512)], rbc[:])

        def emit_proj(cc):
            # v first (its DVE copies clear the queue early), then q/k
            for nt in range(4 * cc, 4 * cc + 4):
                ps = qkps.tile([128, 512], F32, tag="qk", name="qk")
                for dt in range(DT):
                    nc.tensor.matmul(ps[:, 0:256], xt[:, dt, ts(nt, 128)],
                                     wv_sb[:, dt, :],
                                     start=(dt == 0), stop=False)
                nc.tensor.matmul(ps[:, 0:256], msr[:, ts(nt, 128)],
                                 wsv_sb[:, :],
                                 start=False, stop=(not aug))
                if aug:
                    nc.tensor.matmul(ps[:, 0:256], ones_fr[0:1, :], vb_sb[:],
                                     start=False, stop=True)
                nc.vector.tensor_copy(
                    v2[:, nt].rearrange("q p (b c) -> q p b c", b=2)[:, :, :, 0:64],
                    ps[:, 0:256].rearrange("q (p b c) -> q p b c", p=2, b=2))
            for mt in range(4):
                ps = qkps.tile([128, 512], F32, tag="qk", name="qk")
                for dt in range(DT):
                    nc.tensor.matmul(ps[:], wqk_sb[:, dt, ts(mt, 128)],
                                     xt[:, dt, ts(cc, 512)],
                                     start=(dt == 0), stop=False)
                nc.tensor.matmul(ps[:], wsqk_sb[:, ts(mt, 128)],
                                 msr[:, ts(cc, 512)],
                                 start=False, stop=(not aug))
                if aug:
                    nc.tensor.matmul(ps[:], qkb_sb[:, ts(mt, 128)],
                                     ones_row[:], start=False, stop=True)
                nc.scalar.copy(qkT[:, mt, ts(cc, 512)], ps[:])

        # stats run two chunks ahead of projections so the PE never waits
        # on the DVE/ACT row pipeline
        emit_stats(0)
        emit_stats(1)
        emit_norm(0)
        emit_stats(2)
        emit_proj(0)
        emit_norm(1)
        emit_stats(3)
        emit_proj(1)
        emit_norm(2)
        emit_proj(2)
        emit_norm(3)
        emit_proj(3)

        # ---- Phase B: gate = sigmoid(wg @ xn + bg) -------------------------
        for gt in range(2):
            for cc in range(CC):
                ps = qkps.tile([128, 512], F32, tag="qk", name="qk")
                for dt in range(DT):
                    nc.tensor.matmul(ps[:], wg_sb[:, dt, ts(gt, 128)],
                                     xt[:, dt, ts(cc, 512)],
                                     start=(dt == 0), stop=False)
                nc.tensor.matmul(ps[:], wsg_sb[:, ts(gt, 128)],
                                 msr[:, ts(cc, 512)],
                                 start=False, stop=True)
                nc.scalar.activation(g_sb[:, gt, ts(cc, 512)], ps[:],
                                     AF.Sigmoid, bias=bg_sb[:, gt:gt + 1],
                                     scale=1.0)
    xt_stack.close()

    # head-B gate halves moved to partitions 0..63 (for base-0 epilogues)
    nc.sync.dma_start(gB_sb[0:64, :, :], g_sb[64:128, :, :])

    # ---- Phase C: attention -------------------------------------------------
    att = ExitStack()
    den_pool = att.enter_context(tc.tile_pool(name="den", bufs=1))
    ycp_pool = att.enter_context(tc.tile_pool(name="ycp", bufs=4))
    p_pool = att.enter_context(tc.tile_pool(name="pexp", bufs=4))
    row_pool = att.enter_context(tc.tile_pool(name="erow", bufs=2))
    ygt_pool = att.enter_context(tc.tile_pool(name="ygt", bufs=2))
    ot_pool = att.enter_context(tc.tile_pool(name="ot", bufs=1))
    yp_pool = att.enter_context(tc.tile_pool(name="yps", bufs=2, space="PSUM"))

    # per-(chunk, head) denominator row at partition 64, lane-copied from
    # the y accumulator, inverted in place on the DVE (row slice only).
    rd_of = {}
    ycps_of = {}

    def chunk_body(chunk, sps_pool):
        pair, c4 = chunk // 4, chunk % 4
        qmt, kmt = 2 * pair, 2 * pair + 1
        qlo = c4 * 512
        # prefetch next chunks' bias halves
        nxt = chunk + 1
        if nxt < 8 and (nxt, 1) not in bts:
            load_bt(nxt, 1)
        if nxt + 1 < 8 and (nxt + 1, 0) not in bts:
            load_bt(nxt + 1, 0)
        yp = [yp_pool.tile([128, 512], F32, tag="yp", name="yp")
              for _ in range(2)]
        for kt in range(KT):
            half, gi = kt // 8, kt % 8
            bt = bts[(chunk, half)]
            s_ps = sps_pool.tile([128, 1024], F32, tag="sps", name="sps")
            dve_inject = (4 if pair == 0 else 6) <= kt <= 13
            if not dve_inject:
                for hh in range(2):
                    nc.tensor.matmul(s_ps[:, ts(hh, 512)], id_sb[:],
                                     bt[:, gi, hh, :],
                                     start=True, stop=False,
                                     skip_group_check=True)
            for h, base in ((0, 0), (1, 64)):
                nc.tensor.matmul(
                    s_ps[:, ts(h, 512)],
                    qkT[base:base + 64, kmt, ts(kt, 128)],
                    qkT[base:base + 64, qmt, ds(qlo, 512)],
                    start=dve_inject, stop=True, skip_group_check=True)
                if dve_inject:
                    nc.vector.tensor_tensor(
                        out=s_ps[:, ts(h, 512)], in0=s_ps[:, ts(h, 512)],
                        in1=bt[:, gi, h, :], op=OP.add)
            p_t = p_pool.tile([128, 1024], BF16, tag="pt", name="pt")
            nc.scalar.activation(p_t[:], s_ps[:], AF.Exp,
                                 bias=ml_sb[:, kt:kt + 1])
            for h in range(2):
                nc.tensor.matmul(yp[h][0:VW, :],
                                 v2[:, kt, pair, ds(h * VW, VW)],
                                 p_t[:, ts(h, 512)],
                                 start=(kt == 0), stop=(kt == KT - 1))
        del bts[(chunk, 0)], bts[(chunk, 1)]
        ycps, rds = [], []
        for h in range(2):
            ycp = ycp_pool.tile([128, 512], F32R, tag="ycp", name="ycp")
            if h == 1:
                # ACT slots this right before the next chunk's exps,
                # halving the DVE chain that frees the y accumulators
                nc.scalar.copy(ycp[0:64, :], yp[h][0:64, :])
            else:
                nc.vector.tensor_copy(ycp[0:64, :], yp[h][0:64, :])
            # f32 -> f32r copy rounds the denominator row for the PE
            rd = den_pool.tile([128, 512], F32R, tag="rd", name="rd", bufs=6)
            nc.vector.tensor_copy(rd[64:65, :], yp[h][64:65, :])
            ycps.append(ycp)
            rds.append(rd)
        ycps_of[chunk] = ycps
        rd_of[chunk] = rds

    def emit_epilogue(chunk, rb_pool, rb_tag, rb_shape):
        pair, c4 = chunk // 4, chunk % 4
        qlo = c4 * 512
        ycps = ycps_of.pop(chunk)
        rds = rd_of.pop(chunk)
        for h in range(2):
            # broadcast den to 64 partitions, invert on the DVE (free-size
            # bound, so the [64,512] reciprocal costs the same as a row)
            rb = rb_pool.tile(rb_shape, F32, tag=rb_tag, name=rb_tag)
            nc.tensor.matmul(rb[0:64, 0:512],
                             ones_fr[64:65, 0:64],
                             rds[h][64:65, :],
                             start=True, stop=True)
            rden = row_pool.tile([128, 512], F32, tag="rden", name="rden")
            nc.vector.reciprocal_approx_fast(out=rden[0:64, :],
                                             in_=rb[0:64, 0:512])
            gsl = (g_sb if h == 0 else gB_sb)[0:64, pair, ds(qlo, 512)]
            # f32: geff ~ g/den can sit far below the f16 subnormal cutoff
            geff = row_pool.tile([128, 512], F32, tag="geff", name="geff")
            nc.vector.tensor_tensor(out=geff[0:64, :], in0=rden[0:64, :],
                                    in1=gsl, op=OP.mult)
            if h == 0:
                nc.vector.tensor_tensor(out=yg[0:64, pair, ds(qlo, 512)],
                                        in0=ycps[h][0:64, :],
                                        in1=geff[0:64, :], op=OP.mult)
            else:
                ygt = ygt_pool.tile([128, 512], F16, tag="ygt", name="ygt")
                nc.vector.tensor_tensor(out=ygt[0:64, :],
                                        in0=ycps[h][0:64, :],
                                        in1=geff[0:64, :], op=OP.mult)
                nc.sync.dma_start(yg[64:128, pair, ds(qlo, 512)],
                                  ygt[0:64, :])

    def emit_oproj(c4, op_pool, on_act=False):
        ot = ot_pool.tile([128, 4, 1024], F32, tag="ot", name="ot")
        for j, nt in enumerate(range(4 * c4, 4 * c4 + 4)):
            for half in range(2):
                ps = op_pool.tile([128, 512], F32, tag="op", name="op")
                for pt in range(2):
                    nc.tensor.matmul(ps[:], yg[:, pt, ts(nt, 128)],
                                     wo_sb[:, pt, ds(half * 512, 512)],
                                     start=(pt == 0), stop=(pt == 1))
                if on_act:
                    nc.scalar.copy(ot[:, j, ds(half * 512, 512)], ps[:])
                else:
                    nc.vector.tensor_copy(ot[:, j, ds(half * 512, 512)], ps[:])
            nc.sync.dma_start(io["out_p"][c4][:, ds(j * 1024, 1024)],
                              ot[:, j, :])

    # pair 0: deep scores pipeline (3 bufs); epilogues deferred by one
    # chunk, borrowing an sps buffer for the den broadcast.
    with tc.tile_pool(name="sps0", bufs=3, space="PSUM") as sps0:
        for chunk in range(4):
            chunk_body(chunk, sps0)
            if chunk >= 1:
                emit_epilogue(chunk - 1, sps0, "sps", [128, 1024])
    # pair 1: 2-buf scores pipeline + o_proj PSUM; each o_proj group
    # follows the epilogue that completes its yg columns.
    with tc.tile_pool(name="sps1", bufs=2, space="PSUM") as sps1, \
         tc.tile_pool(name="opps", bufs=2, space="PSUM") as opps:
        for chunk in range(4, 8):
            chunk_body(chunk, sps1)
            emit_epilogue(chunk - 1, opps, "op", [128, 512])
            if chunk >= 5:
                emit_oproj(chunk - 5, opps, on_act=(chunk >= 7))
        emit_epilogue(7, opps, "op", [128, 512])
        emit_oproj(3, opps, on_act=True)
    att.close()


def _patched_act_tables():
    """Activation-table selection shim: remove exp/ln/copy/identity/square
    from every set that precedes natural_log_exp_and_others so those
    functions resolve to the combined set (one ACT table load instead of
    thrashing between exp_and_others and natural_log per epilogue).
    Indices/order of the sets are preserved, so the emitted
    act_func_set_id values still point at the canonical act_info.json
    entries and the loaded tables themselves are unchanged."""
    from concourse.hw_specs import get_activation_tables

    strip = {AF.Exp, AF.Ln, AF.Copy, AF.Identity, AF.Square}

    def patched(arch):
        tabs = get_activation_tables(arch)
        if "natural_log_exp_and_others" not in tabs:
            return tabs
        out, seen = {}, False
        for name, fns in tabs.items():
            if name == "natural_log_exp_and_others":
                seen = True
                out[name] = fns
            elif not seen:
                out[name] = fns - strip
            else:
                out[name] = fns
        return out

    return patched


_CACHED = {}


def build_program(aug=False):
    if aug in _CACHED:
        return _CACHED[aug]
    nc = bacc.Bacc("TRN2", target_bir_lowering=False, debug=False,
                   enable_asserts=False, num_devices=NCORES)
    io = {
        "xT": nc.dram_tensor("xT", (CC, 128, DT * 512), F16,
                             kind="ExternalInput").ap(),
        "wqk": nc.dram_tensor("wqk", (D, 512), F16, kind="ExternalInput").ap(),
        "wv": nc.dram_tensor("wv", (D, 256), F16, kind="ExternalInput").ap(),
        "wg": nc.dram_tensor("wg", (D, 256), F16, kind="ExternalInput").ap(),
        "wo": nc.dram_tensor("wo", (256, D), F16, kind="ExternalInput").ap(),
        "bg": nc.dram_tensor("bg", (128, 2), F32, kind="ExternalInput").ap(),
        "maskln": nc.dram_tensor("maskln", (128, KT), F32,
                                 kind="ExternalInput").ap(),
        "wsall": nc.dram_tensor("wsall", (1, 1024), F32R,
                                kind="ExternalInput").ap(),
        "biasT": nc.dram_tensor("biasT", (2, 2, 4, 128, 8 * 2 * 512), F16,
                                kind="ExternalInput").ap(),
        "ident": nc.dram_tensor("ident", (128, 128), F16,
                                kind="ExternalInput").ap(),
        "out_p": nc.dram_tensor("out_p", (CC, 128, 4 * 1024), F32,
                                kind="ExternalOutput").ap(),
    }
    if aug:
        io["qkb"] = nc.dram_tensor("qkb", (1, 512), F32R,
                                   kind="ExternalInput").ap()
        io["vb"] = nc.dram_tensor("vb", (1, 256), F32R,
                                  kind="ExternalInput").ap()

    import concourse.bacc as bacc_mod
    orig = bacc_mod.get_activation_tables
    bacc_mod.get_activation_tables = _patched_act_tables()
    try:
        with tile.TileContext(nc) as tc, ExitStack() as ctx:
            _emit(tc, ctx, io, aug)
        nc.compile()
    finally:
        bacc_mod.get_activation_tables = orig
    _CACHED[aug] = nc
    return nc


def prep_in_maps(x, bias, mask, ln_w, ln_b, w_qkv, w_o, b_o, w_g, b_g):
    """Host-side sharding: slice/transpose/reorder/cast only (plus exact
    folds of ln_w / ln_b / q-scale into weights, which are O(params))."""
    x = np.asarray(x, np.float32)
    bias = np.asarray(bias, np.float32)
    mask = np.asarray(mask)
    ln_w = np.asarray(ln_w, np.float32)
    ln_b = np.asarray(ln_b, np.float32)
    w_qkv = np.asarray(w_qkv, np.float32)
    w_o = np.asarray(w_o, np.float32)
    w_g = np.asarray(w_g, np.float32)
    b_g = np.asarray(b_g, np.float32)

    wql = w_qkv * ln_w[None, :]          # ln_w fold (exact)
    wgl = w_g * ln_w[None, :]
    qkv_lb = w_qkv @ ln_b                # ln_b rank-1 corrections
    g_lb = w_g @ ln_b
    aug = bool(np.any(ln_b != 0))
    qscale = C ** -0.5
    identity = np.eye(128, dtype=np.float16)

    in_maps = []
    for core in range(NCORES):
        b = core // 4
        h0 = HPC * (core % 4)
        # qk weight Mtiles: [qP0, kP0, qP1, kP1], each [A(64)|B(64)] cols
        qk_rows, qk_scale = [], []
        for pair in range(2):
            hA, hB = h0 + 2 * pair, h0 + 2 * pair + 1
            for off, sc in ((0, qscale), (64, 1.0)):
                for h in (hA, hB):
                    qk_rows.extend(range(h * 192 + off, h * 192 + off + 64))
                    qk_scale.extend([sc] * 64)
        qk_rows = np.array(qk_rows)
        qk_scale = np.array(qk_scale, np.float32)
        v_rows = np.concatenate(
            [np.arange(h * 192 + 128, h * 192 + 192) for h in range(h0, h0 + 4)])
        d0 = 64 * h0

        wqk_c = np.ascontiguousarray(
            (wql[qk_rows] * qk_scale[:, None]).T).astype(np.float16)
        wv_c = np.ascontiguousarray(wql[v_rows].T).astype(np.float16)
        wg_c = np.ascontiguousarray(
            wgl[d0:d0 + 256].T).astype(np.float16)
        wo_c = np.ascontiguousarray(w_o[:, d0:d0 + 256].T).astype(np.float16)
        bg_c = np.ascontiguousarray(
            (b_g + g_lb)[d0:d0 + 256].reshape(2, 128).T)
        mf = mask[b].astype(np.float32)
        maskln_c = np.ascontiguousarray(
            np.where(mf == 0, NEG, 0.0).astype(np.float32).reshape(KT, 128).T)
        # biasT host layout [pair, half, c4, p, g, head, q512]
        bb = bias[b, h0:h0 + 4].reshape(2, 2, 4, 512, 2, 8, 128)
        biasT_c = np.ascontiguousarray(
            bb.transpose(0, 4, 2, 6, 5, 1, 3).reshape(
                2, 2, 4, 128, 8 * 2 * 512)).astype(np.float16)
        # xT chunk-blocked: [cc, p, dt, 512]
        xT = x[b].T  # (D, N)
        xT_c = np.ascontiguousarray(
            xT.reshape(DT, 128, CC, 512).transpose(2, 1, 0, 3).reshape(
                CC, 128, DT * 512)).astype(np.float16)
        # colsum corrections of the bf16-rounded weights, so the rank-1
        # mean term matches what the matmuls actually use
        wsall_c = np.ascontiguousarray(np.concatenate(
            [-wqk_c.astype(np.float32).sum(0),
             -wv_c.astype(np.float32).sum(0),
             -wg_c.astype(np.float32).sum(0)]).reshape(1, 1024))

        im = {
            "xT": xT_c, "wqk": wqk_c, "wv": wv_c, "wg": wg_c, "wo": wo_c,
            "bg": bg_c, "maskln": maskln_c,
            "biasT": biasT_c, "ident": identity, "wsall": wsall_c,
        }
        if aug:
            im["qkb"] = np.ascontiguousarray(
                (qkv_lb[qk_rows] * qk_scale).reshape(1, 512).astype(np.float32))
            im["vb"] = np.ascontiguousarray(
                qkv_lb[v_rows].reshape(1, 256).astype(np.float32))
        in_maps.append(im)
    return in_maps


def unshard_out(arr):
    """Device out_p [CC, 128, 4*1024] bf16 -> (N, D) f32 partial."""
    a = np.asarray(arr).astype(np.float32)
    return a.reshape(CC, 128, 4, 1024).transpose(0, 2, 1, 3).reshape(N, D)


def gather(results, b_o):
    b_o = np.asarray(b_o, np.float32)
    out = np.zeros((B, N, D), np.float32)
    for core, res in enumerate(results):
        out[core // 4] += unshard_out(res["out_p"])
    out += b_o[None, None, :]
    return out


def run(inputs, **spmd_kwargs):
    from concourse import bass_utils
    in_maps = prep_in_maps(**inputs)
    nc = build_program(aug="qkb" in in_maps[0])
    res = bass_utils.run_bass_kernel_spmd(
        nc, in_maps, core_ids=list(range(NCORES)), **spmd_kwargs)
    return gather(res.results, inputs["b_o"]), res


def kernel(**inputs) -> np.ndarray:
    out, _ = run(inputs)
    return out
